# revision 9
# baseline (speedup 1.0000x reference)
"""TRN2 Bass kernel for nn_DiffQuantumSimulator (QAOA MaxCut, 18 qubits, p=4).

Strategy: data-parallel over batch (8 graphs -> 8 NeuronCores). Per core the
2^18 statevector lives in SBUF as one [128 x 4096] fp16 tile (re | im halves).

Each QAOA layer applies exp(-i*hp) (diagonal, elementwise) and the mixer
RX(beta)^(x)18 in 3 TensorE matmul phases:
  A: 128x128 complex gate RX^(x)7 on the 7 partition bits, fused with a
     partition<->free-bit transpose by using the *state* as the stationary
     operand (out = state_tile^T @ [C|D]).
  B: same trick on the next 7 bits (stride-16 windows).
  C: standard matmul applying RX^(x)4 (x) I_8 to the remaining 4 bits.
All matmuls run in fp16 (1 col/cycle warm, FWL on LDWEIGHTS). The PE is
pre-warmed with dummy matmuls during the input-DMA head so HAM reaches
K=8/8 before real work starts.

The diagonal rotation (layers 2..4) reads phase C's PSUM output: ScalarE
stages PSUM->SBUF fp16, then VectorE does 4 mult + 2 add/sub in 2x mode.
PSUM->SBUF copies for phases A/B are split across ScalarE/VectorE.

Diagonals (cos/sin of hp per layer, in the layer's bit-layout), the gate
matrices, and hp itself are precomputed on host from the runtime inputs.
Device returns per-partition energy partial sums; host reduces and scales.
"""

import numpy as np

import concourse.bass as bass
import concourse.mybir as mybir
import concourse.tile as tile
from concourse import bacc
from concourse.bass_utils import run_bass_kernel_spmd

N = 18
DIM = 1 << N
P = 128
F = DIM // P  # 2048
LAYERS = 4
BATCH = 8
NCORES = 8

FP32 = mybir.dt.float32
FP16 = mybir.dt.float16
ALU = mybir.AluOpType
ACT = mybir.ActivationFunctionType

# ----------------------------------------------------------------------------
# Host-side math: hp diagonal, gate matrices, bit-layout permutations
# ----------------------------------------------------------------------------


def _compute_hp(adj):
    W = (np.triu(adj, k=1) > 0.5).astype(np.float64)
    n_edges = W.sum()
    idx = np.arange(DIM)
    shifts = (N - 1 - np.arange(N))[:, None]
    Z = 1.0 - 2.0 * ((idx[None, :] >> shifts) & 1).astype(np.float64)
    T = W @ Z
    cross = np.einsum("ud,ud->d", T, Z)
    return 0.5 * (n_edges - cross)  # [DIM], integer-valued*0.5, exact


def _rx(beta):
    c, s = np.cos(beta), np.sin(beta)
    return np.array([[c, -1j * s], [-1j * s, c]], dtype=np.complex128)


def _kron_list(mats):
    out = np.array([[1.0]], dtype=np.complex128)
    for m in mats:
        out = np.kron(out, m)
    return out


def _m7(beta):
    return _kron_list([_rx(beta)] * 7)


def _m41(beta):
    return _kron_list([_rx(beta)] * 4 + [np.eye(2, dtype=np.complex128)] * 3)


def _bitmap_after_A(bm):
    new = [0] * N
    for j in range(7):
        new[11 + j] = bm[j]
    for j in range(4):
        new[7 + j] = bm[7 + j]
    for j in range(7):
        new[j] = bm[11 + j]
    return new


def _bitmap_after_B(bm):
    # window = free bits 10..4 (stride-16 single AP dim), tiles = bits 3..0
    new = [0] * N
    for j in range(7):
        new[11 + j] = bm[4 + j]
    for j in range(4):
        new[7 + j] = bm[j]
    for j in range(7):
        new[j] = bm[11 + j]
    return new


def _perm_for_bitmap(bm):
    a = np.arange(DIM, dtype=np.int64)
    out = np.zeros(DIM, dtype=np.int64)
    for j in range(N):
        out |= ((a >> j) & 1) << bm[j]
    return out


def _layer_perms():
    """Permutations (orig_idx = perm[cur_idx]) for the state layout at the
    start of each layer (1..LAYERS) plus the final layout (index LAYERS)."""
    perms = []
    bm = list(range(N))
    for _ in range(LAYERS):
        perms.append(_perm_for_bitmap(bm))
        bm = _bitmap_after_B(_bitmap_after_A(bm))
    perms.append(_perm_for_bitmap(bm))
    return perms


_PERMS = _layer_perms()


def _host_prep(batch_betas, adj_matrices):
    """Build per-core input dicts."""
    in_maps = []
    for b in range(BATCH):
        hp = _compute_hp(np.asarray(adj_matrices[b], dtype=np.float64))
        cos_hp = np.cos(hp)
        sin_hp = np.sin(hp)

        # init state (= exp(-i hp) applied to unnormalized uniform state),
        # packed [8, P, 512]: chunks 0..3 = re, 4..7 = im
        init = np.empty((8, P, 512), dtype=np.float16)
        re0 = cos_hp[_PERMS[0]].reshape(P, 4, 512)
        im0 = (-sin_hp[_PERMS[0]]).reshape(P, 4, 512)
        for k in range(4):
            init[k] = re0[:, k, :]
            init[4 + k] = im0[:, k, :]

        diags = np.empty((2 * (LAYERS - 1) + 1, P, F), dtype=np.float16)
        for t in range(1, LAYERS):
            diags[2 * (t - 1)] = cos_hp[_PERMS[t]].reshape(P, F)
            diags[2 * (t - 1) + 1] = sin_hp[_PERMS[t]].reshape(P, F)
        diags[-1] = hp[_PERMS[LAYERS]].reshape(P, F)

        gates_ab = np.empty((LAYERS, P, 512), dtype=np.float16)
        gates_c = np.empty((LAYERS, P, 384), dtype=np.float16)
        for t in range(LAYERS):
            beta = float(np.asarray(batch_betas[b][t], dtype=np.float64))
            M7 = _m7(beta)
            C7 = M7.real
            D7 = M7.imag
            M41 = _m41(beta)
            C41 = M41.real
            D41 = M41.imag
            gates_ab[t, :, 0:128] = C7
            gates_ab[t, :, 128:256] = D7
            gates_ab[t, :, 256:384] = -D7
            gates_ab[t, :, 384:512] = C7
            gates_c[t, :, 0:128] = C41
            gates_c[t, :, 128:256] = -D41
            gates_c[t, :, 256:384] = D41

        in_maps.append(
            {
                "init": init,
                "diags": diags,
                "gates_ab": gates_ab,
                "gates_c": gates_c,
            }
        )
    return in_maps


# ----------------------------------------------------------------------------
# Bass program
# ----------------------------------------------------------------------------

N_WARMUP_MM = 12  # dummy matmuls (N=512) to warm HAM during the DMA head
ROT_ADD_GPSIMD = False  # GpSimd TT contends with DVE SBUF ports: net loss


def _build_program():
    nc = bacc.Bacc("TRN2", target_bir_lowering=False, debug=False)

    d_init = nc.dram_tensor("init", [8, P, 512], FP16, kind="ExternalInput")
    n_diag = 2 * (LAYERS - 1) + 1
    d_diags = nc.dram_tensor("diags", [n_diag, P, F], FP16, kind="ExternalInput")
    d_gab = nc.dram_tensor("gates_ab", [LAYERS, P, 512], FP16, kind="ExternalInput")
    d_gc = nc.dram_tensor("gates_c", [LAYERS, P, 384], FP16, kind="ExternalInput")
    d_out = nc.dram_tensor("out", [P, 1], FP32, kind="ExternalOutput")

    with tile.TileContext(nc) as tc:
        with (
            tc.tile_pool(name="state", bufs=1) as st_pool,
            tc.tile_pool(name="consts", bufs=1) as c_pool,
            tc.tile_pool(name="rot", bufs=2) as r_pool,
            tc.tile_pool(name="scratch", bufs=1) as s_pool,
            tc.tile_pool(name="ps_mm", bufs=2, space="PSUM") as ps_mm,
            tc.tile_pool(name="ps_c", bufs=3, space="PSUM") as ps_c,
        ):
            # state tiles: [re(2048) | im(2048)]
            st_a = st_pool.tile([P, 2 * F], FP16, tag="st_a")
            st_b = st_pool.tile([P, 2 * F], FP16, tag="st_b")

            diag_t = [
                c_pool.tile([P, F], FP16, tag=f"diag{k}", name=f"diag{k}")
                for k in range(n_diag)
            ]
            gab_t = [
                c_pool.tile([P, 512], FP16, tag=f"gab{t}", name=f"gab{t}")
                for t in range(LAYERS)
            ]
            gc_t = [
                c_pool.tile([P, 384], FP16, tag=f"gc{t}", name=f"gc{t}")
                for t in range(LAYERS)
            ]

            # rotation scratch (per chunk, double-buffered via pool bufs)
            wu = s_pool.tile([P, 512], FP16, tag="wu")
            sq_re = s_pool.tile([P, 512], FP16, tag="sq_re")
            sq_im = s_pool.tile([P, 512], FP16, tag="sq_im")
            probs = s_pool.tile([P, 512], FP16, tag="probs")
            part_k = [
                s_pool.tile([P, 1], FP32, tag=f"part{k}", name=f"part{k}")
                for k in range(4)
            ]
            comb = [
                s_pool.tile([P, 1], FP32, tag=f"comb{k}", name=f"comb{k}")
                for k in range(3)
            ]

            # ---- PE warmup: dummy matmuls on zeroed SBUF during DMA head
            nc.vector.memset(wu[:], 0.0)
            for i in range(N_WARMUP_MM):
                wu_ps = ps_c.tile([P, 1024], FP32, tag="ps_c", name=f"wu{i}")
                nc.tensor.matmul(
                    wu_ps[:, 0:512], wu[:, 0:128], wu[:], start=True, stop=True
                )

            # ---- input DMAs. NONE on scalar/vector (their queues must stay
            # free for compute). sync = HWDGE (cheap issue), gpsimd = SWDGE.
            # critical order: layer-1 gates + init first, then diags by use.
            nc.sync.dma_start(gab_t[0][:], d_gab.ap()[0])
            nc.sync.dma_start(gc_t[0][:], d_gc.ap()[0])
            for k in range(4):
                nc.sync.dma_start(st_a[:, 512 * k : 512 * (k + 1)], d_init.ap()[k])
                nc.sync.dma_start(
                    st_a[:, 2048 + 512 * k : 2048 + 512 * (k + 1)], d_init.ap()[4 + k]
                )
            nc.sync.dma_start(gab_t[1][:], d_gab.ap()[1])
            nc.sync.dma_start(gab_t[2][:], d_gab.ap()[2])
            nc.sync.dma_start(gab_t[3][:], d_gab.ap()[3])
            # layer-2 rotation diags, halved for earlier first-chunk arrival
            nc.gpsimd.dma_start(diag_t[0][:, 0:1024], d_diags.ap()[0][:, 0:1024])
            nc.gpsimd.dma_start(diag_t[1][:, 0:1024], d_diags.ap()[1][:, 0:1024])
            nc.gpsimd.dma_start(diag_t[0][:, 1024:2048], d_diags.ap()[0][:, 1024:2048])
            nc.gpsimd.dma_start(diag_t[1][:, 1024:2048], d_diags.ap()[1][:, 1024:2048])
            nc.gpsimd.dma_start(gc_t[1][:], d_gc.ap()[1])
            nc.gpsimd.dma_start(diag_t[2][:], d_diags.ap()[2])  # cos3
            nc.gpsimd.dma_start(diag_t[3][:], d_diags.ap()[3])  # sin3
            nc.gpsimd.dma_start(gc_t[2][:], d_gc.ap()[2])
            nc.gpsimd.dma_start(diag_t[4][:], d_diags.ap()[4])  # cos4
            nc.gpsimd.dma_start(diag_t[5][:], d_diags.ap()[5])  # sin4
            nc.gpsimd.dma_start(gc_t[3][:], d_gc.ap()[3])
            nc.gpsimd.dma_start(diag_t[6][:], d_diags.ap()[6])  # hp (energy)

            def joined_view(tile_, g):
                # [P, j(2), c(re/im 2), h(128)] view of dst cols
                # c*2048 + 256*g + 128*j + h
                v = tile_[:].rearrange(
                    "p (c g j h) -> p g j c h", c=2, g=8, j=2
                )
                return v[:, g]

            def copy_group(engine, dst_tile, g, ps):
                src = ps[:].rearrange("p (j c h) -> p j c h", j=2, c=2)
                dst = joined_view(dst_tile, g)
                if engine == "v":
                    nc.vector.tensor_copy(dst, src)
                else:
                    nc.scalar.copy(dst, src)

            def a_group(t, g, src_tile, dst_tile):
                """Phase A/B matmul group g (2 windows) -> ps tile."""
                ps = ps_mm.tile([P, 512], FP32, tag="ps_mm")
                cd7 = gab_t[t][:, 0:256]
                ndc7 = gab_t[t][:, 256:512]
                for j in range(2):
                    w = 2 * g + j
                    out_sl = ps[:, 256 * j : 256 * (j + 1)]
                    re_w = src_tile[:, 128 * w : 128 * (w + 1)]
                    im_w = src_tile[:, 2048 + 128 * w : 2048 + 128 * (w + 1)]
                    nc.tensor.matmul(out_sl, re_w, cd7, start=True, stop=False)
                    nc.tensor.matmul(out_sl, im_w, ndc7, start=False, stop=True)
                return ps

            def b_group(t, g, src_tile):
                ps = ps_mm.tile([P, 512], FP32, tag="ps_mm")
                cd7 = gab_t[t][:, 0:256]
                ndc7 = gab_t[t][:, 256:512]
                sv = src_tile[:].rearrange("p (c x u) -> p c x u", c=2, x=128)
                for j in range(2):
                    w = 2 * g + j
                    out_sl = ps[:, 256 * j : 256 * (j + 1)]
                    nc.tensor.matmul(out_sl, sv[:, 0, :, w], cd7, start=True, stop=False)
                    nc.tensor.matmul(out_sl, sv[:, 1, :, w], ndc7, start=False, stop=True)
                return ps

            def c_chunk(t, k, src_tile):
                """Phase C chunk k: pc = [pre(512) | pim(512)]."""
                pc = ps_c.tile([P, 1024], FP32, tag="ps_c", name=f"pc{t}{k}")
                c41 = gc_t[t][:, 0:128]
                nd41 = gc_t[t][:, 128:256]
                d41 = gc_t[t][:, 256:384]
                ck_re = src_tile[:, 512 * k : 512 * (k + 1)]
                ck_im = src_tile[:, 2048 + 512 * k : 2048 + 512 * (k + 1)]
                pre = pc[:, 0:512]
                pim = pc[:, 512:1024]
                # LDW-minimizing order: c41 used by both accumulation groups
                nc.tensor.matmul(pre, c41, ck_re, start=True, stop=False)
                nc.tensor.matmul(pim, c41, ck_im, start=True, stop=False)
                nc.tensor.matmul(pre, nd41, ck_im, start=False, stop=True)
                nc.tensor.matmul(pim, d41, ck_re, start=False, stop=True)
                return pc

            def rot_chunk(t, k, pc, dst_tile):
                """Apply exp(-i hp) to C output chunk k -> dst state chunk k."""
                cos_d = diag_t[2 * (t - 1)][:, 512 * k : 512 * (k + 1)]
                sin_d = diag_t[2 * (t - 1) + 1][:, 512 * k : 512 * (k + 1)]
                sc = r_pool.tile([P, 1024], FP16, tag="sc")
                nc.scalar.copy(sc[:], pc[:])
                s_re = sc[:, 0:512]
                s_im = sc[:, 512:1024]
                rs0 = r_pool.tile([P, 512], FP16, tag="rs0")
                rs1 = r_pool.tile([P, 512], FP16, tag="rs1")
                rs2 = r_pool.tile([P, 512], FP16, tag="rs2")
                rs3 = r_pool.tile([P, 512], FP16, tag="rs3")
                nc.vector.tensor_tensor(rs0[:], s_re, cos_d, ALU.mult)
                nc.vector.tensor_tensor(rs1[:], s_im, sin_d, ALU.mult)
                nc.vector.tensor_tensor(rs2[:], s_re, sin_d, ALU.mult)
                nc.vector.tensor_tensor(rs3[:], s_im, cos_d, ALU.mult)
                dst_re = dst_tile[:, 512 * k : 512 * (k + 1)]
                dst_im = dst_tile[:, 2048 + 512 * k : 2048 + 512 * (k + 1)]
                add_eng = nc.gpsimd if ROT_ADD_GPSIMD else nc.vector
                add_eng.tensor_tensor(dst_re, rs0[:], rs1[:], ALU.add)
                add_eng.tensor_tensor(dst_im, rs3[:], rs2[:], ALU.subtract)

            # copy-engine assignment: "v" = vector, "s" = scalar
            # layers with rotation load DVE heavily -> scalar takes more copies
            A_ENG = {
                0: ["v", "s", "v", "s", "v", "s", "v", "s"],
                1: ["s", "v", "s", "s", "v", "s", "s", "v"],
            }
            B_ENG = {
                0: ["v", "s", "v", "s", "v", "s", "v", "s"],
                1: ["s", "v", "s", "s", "v", "s", "s", "v"],
            }

            hp_d = diag_t[n_diag - 1]

            def energy_chunk(k, pc):
                ck = slice(512 * k, 512 * (k + 1))
                nc.scalar.activation(sq_re[:], pc[:, 0:512], ACT.Square)
                nc.scalar.activation(sq_im[:], pc[:, 512:1024], ACT.Square)
                nc.vector.tensor_tensor(probs[:], sq_re[:], sq_im[:], ALU.add)
                nc.vector.scalar_tensor_tensor(
                    sq_re[:],  # dummy out, reused
                    probs[:],
                    1.0,
                    hp_d[:, ck],
                    ALU.mult,
                    ALU.mult,
                    accum_out=part_k[k][:],
                )

            for t in range(LAYERS):
                a_eng = A_ENG[min(t, 1)]
                b_eng = B_ENG[min(t, 1)]
                # ---- phase A (st_a ready: t=0 from DMA, else from the
                # rotation emitted inside layer t-1's C loop)
                for g in range(8):
                    ps = a_group(t, g, st_a, st_b)
                    copy_group(a_eng[g], st_b, g, ps)
                # ---- phase B
                for g in range(8):
                    ps = b_group(t, g, st_b)
                    copy_group(b_eng[g], st_a, g, ps)
                # ---- phase C, fused with next layer's rotation (or the
                # energy reduction) per chunk so it overlaps the C matmuls
                for k in range(4):
                    pc = c_chunk(t, k, st_a)
                    if t < LAYERS - 1:
                        rot_chunk(t + 1, k, pc, st_a)
                    else:
                        energy_chunk(k, pc)
            nc.vector.tensor_tensor(comb[0][:], part_k[0][:], part_k[1][:], ALU.add)
            nc.vector.tensor_tensor(comb[1][:], part_k[2][:], part_k[3][:], ALU.add)
            nc.vector.tensor_tensor(comb[2][:], comb[0][:], comb[1][:], ALU.add)
            nc.sync.dma_start(d_out.ap(), comb[2][:])

    nc.compile()
    return nc


_NC_CACHE = {}


def _get_program():
    if "nc" not in _NC_CACHE:
        _NC_CACHE["nc"] = _build_program()
    return _NC_CACHE["nc"]


def kernel(batch_betas, adj_matrices, _trace=False, _tmpdir=None):
    batch_betas = np.asarray(batch_betas, dtype=np.float32)
    adj_matrices = np.asarray(adj_matrices, dtype=np.float32)
    assert batch_betas.shape == (BATCH, LAYERS)
    assert adj_matrices.shape == (BATCH, N, N)

    nc = _get_program()
    in_maps = _host_prep(batch_betas, adj_matrices)
    res = run_bass_kernel_spmd(
        nc,
        in_maps,
        list(range(NCORES)),
        trace=_trace,
        tmpdir=_tmpdir,
    )
    energies = np.array(
        [res.results[b]["out"].sum() / DIM for b in range(BATCH)], dtype=np.float32
    )
    if _trace:
        return energies, res
    return energies


# revision 12
# speedup vs baseline: 1.1651x; 1.1651x over previous
"""TRN2 Bass kernel for nn_DiffQuantumSimulator (QAOA MaxCut, 18 qubits, p=4).

Strategy: data-parallel over batch (8 graphs -> 8 NeuronCores). Per core the
2^18 statevector lives in SBUF as one [128 x 4096] fp16 tile (re | im halves).

Each QAOA layer applies exp(-i*hp) (diagonal, elementwise) and the mixer
RX(beta)^(x)18 in 3 TensorE matmul phases:
  A: 128x128 complex gate RX^(x)7 on the 7 partition bits, fused with a
     partition<->free-bit transpose by using the *state* as the stationary
     operand (out = state_tile^T @ [C|D]).
  B: same trick on the next 7 bits (stride-16 windows).
  C: standard matmul applying RX^(x)4 (x) I_8 to the remaining 4 bits.
All matmuls run in fp16 (1 col/cycle warm, FWL on LDWEIGHTS). The PE is
pre-warmed with dummy matmuls during the input-DMA head so HAM reaches
K=8/8 before real work starts.

The diagonal rotation (layers 2..4) reads phase C's PSUM output: ScalarE
stages PSUM->SBUF fp16, then VectorE does 4 mult + 2 add/sub in 2x mode.
PSUM->SBUF copies for phases A/B are split across ScalarE/VectorE.

Diagonals (cos/sin of hp per layer, in the layer's bit-layout), the gate
matrices, and hp itself are precomputed on host from the runtime inputs.
Device returns per-partition energy partial sums; host reduces and scales.
"""

import numpy as np

import concourse.bass as bass
import concourse.mybir as mybir
import concourse.tile as tile
from concourse import bacc
from concourse.bass_utils import run_bass_kernel_spmd

N = 18
DIM = 1 << N
P = 128
F = DIM // P  # 2048
LAYERS = 4
BATCH = 8
NCORES = 8

FP32 = mybir.dt.float32
FP16 = mybir.dt.float16
ALU = mybir.AluOpType
ACT = mybir.ActivationFunctionType

# ----------------------------------------------------------------------------
# Host-side math: hp diagonal, gate matrices, bit-layout permutations
# ----------------------------------------------------------------------------


def _compute_hp(adj):
    W = (np.triu(adj, k=1) > 0.5).astype(np.float64)
    n_edges = W.sum()
    idx = np.arange(DIM)
    shifts = (N - 1 - np.arange(N))[:, None]
    Z = 1.0 - 2.0 * ((idx[None, :] >> shifts) & 1).astype(np.float64)
    T = W @ Z
    cross = np.einsum("ud,ud->d", T, Z)
    return 0.5 * (n_edges - cross)  # [DIM], integer-valued*0.5, exact


def _rx(beta):
    c, s = np.cos(beta), np.sin(beta)
    return np.array([[c, -1j * s], [-1j * s, c]], dtype=np.complex128)


def _kron_list(mats):
    out = np.array([[1.0]], dtype=np.complex128)
    for m in mats:
        out = np.kron(out, m)
    return out


def _m7(beta):
    return _kron_list([_rx(beta)] * 7)


def _m41(beta):
    return _kron_list([_rx(beta)] * 4 + [np.eye(2, dtype=np.complex128)] * 3)


def _bitmap_after_A(bm):
    new = [0] * N
    for j in range(7):
        new[11 + j] = bm[j]
    for j in range(4):
        new[7 + j] = bm[7 + j]
    for j in range(7):
        new[j] = bm[11 + j]
    return new


def _bitmap_after_B(bm):
    # window = free bits 10..4 (stride-16 single AP dim), tiles = bits 3..0
    new = [0] * N
    for j in range(7):
        new[11 + j] = bm[4 + j]
    for j in range(4):
        new[7 + j] = bm[j]
    for j in range(7):
        new[j] = bm[11 + j]
    return new


def _perm_for_bitmap(bm):
    a = np.arange(DIM, dtype=np.int64)
    out = np.zeros(DIM, dtype=np.int64)
    for j in range(N):
        out |= ((a >> j) & 1) << bm[j]
    return out


def _layer_perms():
    """Permutations (orig_idx = perm[cur_idx]) for the state layout at the
    start of each layer (1..LAYERS) plus the final layout (index LAYERS)."""
    perms = []
    bm = list(range(N))
    for _ in range(LAYERS):
        perms.append(_perm_for_bitmap(bm))
        bm = _bitmap_after_B(_bitmap_after_A(bm))
    perms.append(_perm_for_bitmap(bm))
    return perms


_PERMS = _layer_perms()


def _host_prep(batch_betas, adj_matrices):
    """Build per-core input dicts."""
    in_maps = []
    for b in range(BATCH):
        hp = _compute_hp(np.asarray(adj_matrices[b], dtype=np.float64))
        cos_hp = np.cos(hp)
        sin_hp = np.sin(hp)

        # init state (= exp(-i hp) applied to unnormalized uniform state),
        # packed [8, P, 512]: chunks 0..3 = re, 4..7 = im
        init = np.empty((8, P, 512), dtype=np.float16)
        re0 = cos_hp[_PERMS[0]].reshape(P, 4, 512)
        im0 = (-sin_hp[_PERMS[0]]).reshape(P, 4, 512)
        for k in range(4):
            init[k] = re0[:, k, :]
            init[4 + k] = im0[:, k, :]

        diags = np.empty((2 * (LAYERS - 1) + 1, P, F), dtype=np.float16)
        for t in range(1, LAYERS):
            diags[2 * (t - 1)] = cos_hp[_PERMS[t]].reshape(P, F)
            diags[2 * (t - 1) + 1] = sin_hp[_PERMS[t]].reshape(P, F)
        diags[-1] = hp[_PERMS[LAYERS]].reshape(P, F)

        gates_ab = np.empty((LAYERS, P, 512), dtype=np.float16)
        gates_c = np.empty((LAYERS, P, 384), dtype=np.float16)
        for t in range(LAYERS):
            beta = float(np.asarray(batch_betas[b][t], dtype=np.float64))
            M7 = _m7(beta)
            C7 = M7.real
            D7 = M7.imag
            M41 = _m41(beta)
            C41 = M41.real
            D41 = M41.imag
            gates_ab[t, :, 0:128] = C7
            gates_ab[t, :, 128:256] = D7
            gates_ab[t, :, 256:384] = -D7
            gates_ab[t, :, 384:512] = C7
            gates_c[t, :, 0:128] = C41
            gates_c[t, :, 128:256] = -D41
            gates_c[t, :, 256:384] = D41

        in_maps.append(
            {
                "init": init,
                "diags": diags,
                "gates_ab": gates_ab,
                "gates_c": gates_c,
            }
        )
    return in_maps


# ----------------------------------------------------------------------------
# Bass program
# ----------------------------------------------------------------------------

N_WARMUP_MM = 12  # dummy matmuls (N=512) to warm HAM during the DMA head
ROT_ADD_GPSIMD = False  # GpSimd TT contends with DVE SBUF ports: net loss


def _build_program():
    nc = bacc.Bacc("TRN2", target_bir_lowering=False, debug=False)

    d_init = nc.dram_tensor("init", [8, P, 512], FP16, kind="ExternalInput")
    n_diag = 2 * (LAYERS - 1) + 1
    d_diags = nc.dram_tensor("diags", [n_diag, P, F], FP16, kind="ExternalInput")
    d_gab = nc.dram_tensor("gates_ab", [LAYERS, P, 512], FP16, kind="ExternalInput")
    d_gc = nc.dram_tensor("gates_c", [LAYERS, P, 384], FP16, kind="ExternalInput")
    d_out = nc.dram_tensor("out", [P, 1], FP32, kind="ExternalOutput")

    with tile.TileContext(nc) as tc:
        with (
            tc.tile_pool(name="state", bufs=1) as st_pool,
            tc.tile_pool(name="consts", bufs=1) as c_pool,
            tc.tile_pool(name="rot", bufs=2) as r_pool,
            tc.tile_pool(name="scratch", bufs=1) as s_pool,
            tc.tile_pool(name="ps_mm", bufs=2, space="PSUM") as ps_mm,
            tc.tile_pool(name="ps_c", bufs=2, space="PSUM") as ps_c,
            tc.tile_pool(name="ps_d", bufs=1, space="PSUM") as ps_d,
        ):
            # state tiles: [re(2048) | im(2048)]
            st_a = st_pool.tile([P, 2 * F], FP16, tag="st_a")
            st_b = st_pool.tile([P, 2 * F], FP16, tag="st_b")

            diag_t = [
                c_pool.tile([P, F], FP16, tag=f"diag{k}", name=f"diag{k}")
                for k in range(n_diag)
            ]
            gab_t = [
                c_pool.tile([P, 512], FP16, tag=f"gab{t}", name=f"gab{t}")
                for t in range(LAYERS)
            ]
            gc_t = [
                c_pool.tile([P, 384], FP16, tag=f"gc{t}", name=f"gc{t}")
                for t in range(LAYERS)
            ]

            # rotation scratch (per chunk, double-buffered via pool bufs)
            wu = s_pool.tile([P, 512], FP16, tag="wu")
            sq_re = s_pool.tile([P, 512], FP16, tag="sq_re")
            sq_im = s_pool.tile([P, 512], FP16, tag="sq_im")
            probs = s_pool.tile([P, 512], FP16, tag="probs")
            part_k = [
                s_pool.tile([P, 1], FP32, tag=f"part{k}", name=f"part{k}")
                for k in range(4)
            ]
            comb = [
                s_pool.tile([P, 1], FP32, tag=f"comb{k}", name=f"comb{k}")
                for k in range(3)
            ]

            # ---- PE warmup / HAM-hold fillers: dummy matmuls on zeroed SBUF
            # into a dedicated PSUM bank (no deps on real work)
            nc.vector.memset(wu[:], 0.0)
            dps = ps_d.tile([P, 512], FP32, tag="ps_d")

            def dummy_mms(n, cols=256):
                for _ in range(n):
                    nc.tensor.matmul(
                        dps[:, 0:cols], wu[:, 0:128], wu[:, 0:cols],
                        start=True, stop=True,
                    )

            dummy_mms(N_WARMUP_MM, cols=512)

            # ---- input DMAs. NONE on scalar/vector (their queues must stay
            # free for compute). sync = HWDGE (cheap issue), gpsimd = SWDGE.
            # critical order: layer-1 gates + init first, then diags by use.
            nc.sync.dma_start(gab_t[0][:], d_gab.ap()[0])
            nc.sync.dma_start(gc_t[0][:], d_gc.ap()[0])
            for k in range(4):
                nc.sync.dma_start(st_a[:, 512 * k : 512 * (k + 1)], d_init.ap()[k])
                nc.sync.dma_start(
                    st_a[:, 2048 + 512 * k : 2048 + 512 * (k + 1)], d_init.ap()[4 + k]
                )
            nc.sync.dma_start(gab_t[1][:], d_gab.ap()[1])
            nc.sync.dma_start(gab_t[2][:], d_gab.ap()[2])
            nc.sync.dma_start(gab_t[3][:], d_gab.ap()[3])
            # layer-2 rotation diags, halved for earlier first-chunk arrival
            nc.gpsimd.dma_start(diag_t[0][:, 0:1024], d_diags.ap()[0][:, 0:1024])
            nc.gpsimd.dma_start(diag_t[1][:, 0:1024], d_diags.ap()[1][:, 0:1024])
            nc.gpsimd.dma_start(diag_t[0][:, 1024:2048], d_diags.ap()[0][:, 1024:2048])
            nc.gpsimd.dma_start(diag_t[1][:, 1024:2048], d_diags.ap()[1][:, 1024:2048])
            nc.gpsimd.dma_start(gc_t[1][:], d_gc.ap()[1])
            nc.gpsimd.dma_start(diag_t[2][:], d_diags.ap()[2])  # cos3
            nc.gpsimd.dma_start(diag_t[3][:], d_diags.ap()[3])  # sin3
            nc.gpsimd.dma_start(gc_t[2][:], d_gc.ap()[2])
            nc.gpsimd.dma_start(diag_t[4][:], d_diags.ap()[4])  # cos4
            nc.gpsimd.dma_start(diag_t[5][:], d_diags.ap()[5])  # sin4
            nc.gpsimd.dma_start(gc_t[3][:], d_gc.ap()[3])
            nc.gpsimd.dma_start(diag_t[6][:], d_diags.ap()[6])  # hp (energy)

            def joined_view(tile_, g):
                # [P, j(2), c(re/im 2), h(128)] view of dst cols
                # c*2048 + 256*g + 128*j + h
                v = tile_[:].rearrange(
                    "p (c g j h) -> p g j c h", c=2, g=8, j=2
                )
                return v[:, g]

            def copy_group(engine, dst_tile, g, ps):
                src = ps[:].rearrange("p (j c h) -> p j c h", j=2, c=2)
                dst = joined_view(dst_tile, g)
                if engine == "v":
                    nc.vector.tensor_copy(dst, src)
                else:
                    nc.scalar.copy(dst, src)

            def a_group(t, g, src_tile, dst_tile):
                """Phase A/B matmul group g (2 windows) -> ps tile."""
                ps = ps_mm.tile([P, 512], FP32, tag="ps_mm")
                cd7 = gab_t[t][:, 0:256]
                ndc7 = gab_t[t][:, 256:512]
                for j in range(2):
                    w = 2 * g + j
                    out_sl = ps[:, 256 * j : 256 * (j + 1)]
                    re_w = src_tile[:, 128 * w : 128 * (w + 1)]
                    im_w = src_tile[:, 2048 + 128 * w : 2048 + 128 * (w + 1)]
                    nc.tensor.matmul(out_sl, re_w, cd7, start=True, stop=False)
                    nc.tensor.matmul(out_sl, im_w, ndc7, start=False, stop=True)
                return ps

            def b_group(t, g, src_tile):
                ps = ps_mm.tile([P, 512], FP32, tag="ps_mm")
                cd7 = gab_t[t][:, 0:256]
                ndc7 = gab_t[t][:, 256:512]
                sv = src_tile[:].rearrange("p (c x u) -> p c x u", c=2, x=128)
                for j in range(2):
                    w = 2 * g + j
                    out_sl = ps[:, 256 * j : 256 * (j + 1)]
                    nc.tensor.matmul(out_sl, sv[:, 0, :, w], cd7, start=True, stop=False)
                    nc.tensor.matmul(out_sl, sv[:, 1, :, w], ndc7, start=False, stop=True)
                return ps

            def c_chunk(t, k, src_tile):
                """Phase C chunk k: pc = [pre(512) | pim(512)]."""
                pc = ps_c.tile([P, 1024], FP32, tag="ps_c", name=f"pc{t}{k}")
                c41 = gc_t[t][:, 0:128]
                nd41 = gc_t[t][:, 128:256]
                d41 = gc_t[t][:, 256:384]
                ck_re = src_tile[:, 512 * k : 512 * (k + 1)]
                ck_im = src_tile[:, 2048 + 512 * k : 2048 + 512 * (k + 1)]
                pre = pc[:, 0:512]
                pim = pc[:, 512:1024]
                # LDW-minimizing order: c41 used by both accumulation groups
                nc.tensor.matmul(pre, c41, ck_re, start=True, stop=False)
                nc.tensor.matmul(pim, c41, ck_im, start=True, stop=False)
                nc.tensor.matmul(pre, nd41, ck_im, start=False, stop=True)
                nc.tensor.matmul(pim, d41, ck_re, start=False, stop=True)
                return pc

            def rot_chunk(t, k, pc, dst_tile):
                """Apply exp(-i hp) to C output chunk k -> dst state chunk k."""
                cos_d = diag_t[2 * (t - 1)][:, 512 * k : 512 * (k + 1)]
                sin_d = diag_t[2 * (t - 1) + 1][:, 512 * k : 512 * (k + 1)]
                sc = r_pool.tile([P, 1024], FP16, tag="sc")
                nc.scalar.copy(sc[:], pc[:])
                s_re = sc[:, 0:512]
                s_im = sc[:, 512:1024]
                rs0 = r_pool.tile([P, 512], FP16, tag="rs0")
                rs1 = r_pool.tile([P, 512], FP16, tag="rs1")
                rs2 = r_pool.tile([P, 512], FP16, tag="rs2")
                rs3 = r_pool.tile([P, 512], FP16, tag="rs3")
                nc.vector.tensor_tensor(rs0[:], s_re, cos_d, ALU.mult)
                nc.vector.tensor_tensor(rs1[:], s_im, sin_d, ALU.mult)
                nc.vector.tensor_tensor(rs2[:], s_re, sin_d, ALU.mult)
                nc.vector.tensor_tensor(rs3[:], s_im, cos_d, ALU.mult)
                dst_re = dst_tile[:, 512 * k : 512 * (k + 1)]
                dst_im = dst_tile[:, 2048 + 512 * k : 2048 + 512 * (k + 1)]
                add_eng = nc.gpsimd if ROT_ADD_GPSIMD else nc.vector
                add_eng.tensor_tensor(dst_re, rs0[:], rs1[:], ALU.add)
                add_eng.tensor_tensor(dst_im, rs3[:], rs2[:], ALU.subtract)

            # copy-engine assignment: "v" = vector, "s" = scalar
            # layers with rotation load DVE heavily -> scalar takes more copies
            A_ENG = {
                0: ["v", "s", "v", "s", "v", "s", "v", "s"],
                1: ["s", "v", "s", "s", "v", "s", "s", "v"],
            }
            B_ENG = {
                0: ["v", "s", "v", "s", "v", "s", "v", "s"],
                1: ["s", "v", "s", "s", "v", "s", "s", "v"],
            }

            hp_d = diag_t[n_diag - 1]

            def energy_chunk(k, pc):
                ck = slice(512 * k, 512 * (k + 1))
                nc.scalar.activation(sq_re[:], pc[:, 0:512], ACT.Square)
                nc.scalar.activation(sq_im[:], pc[:, 512:1024], ACT.Square)
                nc.vector.tensor_tensor(probs[:], sq_re[:], sq_im[:], ALU.add)
                nc.vector.scalar_tensor_tensor(
                    sq_re[:],  # dummy out, reused
                    probs[:],
                    1.0,
                    hp_d[:, ck],
                    ALU.mult,
                    ALU.mult,
                    accum_out=part_k[k][:],
                )

            for t in range(LAYERS):
                a_eng = A_ENG[min(t, 1)]
                b_eng = B_ENG[min(t, 1)]
                # ---- phase A (st_a ready: t=0 from DMA, else from the
                # rotation emitted inside layer t-1's B/C loop)
                for g in range(8):
                    ps = a_group(t, g, st_a, st_b)
                    copy_group(a_eng[g], st_b, g, ps)
                # hold HAM warm while the last A copies land (A->B barrier)
                dummy_mms(6)
                # ---- phase B interleaved with phase C + rotation: C chunk k
                # needs only B groups 2k,2k+1, so emit it right after them --
                # this starts the rotation chain as early as possible and
                # spreads it across the whole B+C matmul stretch.
                for k in range(4):
                    for g in (2 * k, 2 * k + 1):
                        ps = b_group(t, g, st_b)
                        copy_group(b_eng[g], st_a, g, ps)
                    pc = c_chunk(t, k, st_a)
                    if t < LAYERS - 1:
                        rot_chunk(t + 1, k, pc, st_a)
                    else:
                        energy_chunk(k, pc)
                # hold HAM warm while the first rotation chunk finishes
                if t < LAYERS - 1:
                    dummy_mms(8)
            nc.vector.tensor_tensor(comb[0][:], part_k[0][:], part_k[1][:], ALU.add)
            nc.vector.tensor_tensor(comb[1][:], part_k[2][:], part_k[3][:], ALU.add)
            nc.vector.tensor_tensor(comb[2][:], comb[0][:], comb[1][:], ALU.add)
            nc.sync.dma_start(d_out.ap(), comb[2][:])

    nc.compile()
    return nc


_NC_CACHE = {}


def _get_program():
    if "nc" not in _NC_CACHE:
        _NC_CACHE["nc"] = _build_program()
    return _NC_CACHE["nc"]


def kernel(batch_betas, adj_matrices, _trace=False, _tmpdir=None):
    batch_betas = np.asarray(batch_betas, dtype=np.float32)
    adj_matrices = np.asarray(adj_matrices, dtype=np.float32)
    assert batch_betas.shape == (BATCH, LAYERS)
    assert adj_matrices.shape == (BATCH, N, N)

    nc = _get_program()
    in_maps = _host_prep(batch_betas, adj_matrices)
    res = run_bass_kernel_spmd(
        nc,
        in_maps,
        list(range(NCORES)),
        trace=_trace,
        tmpdir=_tmpdir,
    )
    energies = np.array(
        [res.results[b]["out"].sum() / DIM for b in range(BATCH)], dtype=np.float32
    )
    if _trace:
        return energies, res
    return energies
